# revision 11
# baseline (speedup 1.0000x reference)
"""Trainium2 Bass kernel for FNO1d+LSTM network.

Math (from the reference): the FNO spectral conv has spatial length 1, so the
rfft/irfft collapses to a plain GEMM with spec_wr mode 0 (spec_wi unused):
  h = x @ fc0_w.T + fc0_b
  4x: h = gelu(h @ (spec_wr[l,:,:,0] + lin_w[l].T) + lin_b[l])
  2-layer LSTM (torch gate order i,f,g,o) over S=384 steps
  out = last hidden @ fcout_w.T + fcout_b
Returns (out, stack([h1,h2]), stack([c1,c2])).

Distribution: the sequential LSTM scan dominates, and its per-step matmul
cost is batch-independent (the weight panels are the streamed PE operand),
so batch sharding cannot shorten it.  Every core runs the identical
full-batch problem (pure replication, zero collectives); core 0's output is
returned.

Per-core design:
  FNO stack computed fully transposed (hT[w, s*B+b]) in bf16 SBUF so GELU
  uses all 128 ACT lanes and the per-partition ACT bias slot carries lin_b.
  LSTM scan: 4-way column-tiled matmuls (tile_position=(0,32q)) put the
  gates for hidden quarter q on PSUM partitions [32q,32q+32): partition =
  32q + batch, free = (type,h_lo) with type order (i,f,o,g).  Stationary
  operands are the small [*,32] x/h slices; weight panels stream.  c stays
  fp32 in that gate layout; h is written bf16 and XBAR-DMA-transposed
  ([128,128]) back into lhsT layout for the next step.  The two LSTM layers
  run as a 1-step wavefront so their serial chains overlap.  The main loop
  is a tc.For_i over 16-step bodies; x-slices for the loop body are staged
  into fixed SBUF tiles by a dynamic-offset DMA half a body ahead.
"""

import os
import sys
from contextlib import ExitStack

import numpy as np

if "/opt/trn_rl_repo" not in sys.path:
    sys.path.insert(0, "/opt/trn_rl_repo")

import ml_dtypes

import concourse.bass as bass
from concourse import bacc
import concourse.mybir as mybir
import concourse.tile as tile
from concourse.bass import ds, ts
from concourse.bass_utils import run_bass_kernel_spmd

AF = mybir.ActivationFunctionType
OP = mybir.AluOpType
BF16 = mybir.dt.bfloat16
F32 = mybir.dt.float32

B = 32
D = 64
W = 256
H = 512
G4 = 2048
OUTD = 64
T = int(os.environ.get("KERNEL_T", "384"))
TB = T * B
NCHUNK = TB // 512
NBLK = 4 if NCHUNK % 4 == 0 else 1
N_CORES = int(os.environ.get("KERNEL_N_CORES", "8"))
UNROLL = 16

# gate type order in permuted layout: (i, f, o, g); torch order is (i,f,g,o)
TYP_MAP = [0, 1, 3, 2]
_PERM = np.array(
    [
        TYP_MAP[typ] * 512 + q * 128 + hl
        for q in range(4)
        for typ in range(4)
        for hl in range(128)
    ],
    dtype=np.int64,
)


def _bf(a):
    return np.ascontiguousarray(np.asarray(a, dtype=np.float32).astype(ml_dtypes.bfloat16))


def _f32(a):
    return np.ascontiguousarray(np.asarray(a, dtype=np.float32))


def prepare_inputs(
    x, h0, c0, fc0_w, fc0_b, spec_wr, spec_wi, lin_w, lin_b,
    lstm_wih0, lstm_whh0, lstm_bih0, lstm_bhh0,
    lstm_wih1, lstm_whh1, lstm_bih1, lstm_bhh1, fcout_w, fcout_b,
):
    x = np.asarray(x, dtype=np.float32)
    xT = x.transpose(2, 1, 0).reshape(D, TB)  # col = s*B + b
    fno_w = np.stack(
        [np.asarray(spec_wr)[l, :, :, 0] + np.asarray(lin_w)[l].T for l in range(4)]
    )
    fnoW = fno_w.reshape(4, 2, 128, 2, 128).transpose(0, 1, 3, 2, 4)  # [l,kt,mt,128,128]
    linb = np.asarray(lin_b, dtype=np.float32).reshape(4, 2, 128)

    W0 = np.concatenate([np.asarray(lstm_wih0).T, np.asarray(lstm_whh0).T], 0)[:, _PERM]
    b0 = (np.asarray(lstm_bih0) + np.asarray(lstm_bhh0))[_PERM]
    W1 = np.concatenate([np.asarray(lstm_wih1).T, np.asarray(lstm_whh1).T], 0)[:, _PERM]
    b1 = (np.asarray(lstm_bih1) + np.asarray(lstm_bhh1))[_PERM]

    fcoutT = np.asarray(fcout_w).T.reshape(4, 128, OUTD).transpose(1, 0, 2)  # [128,4,64]

    return {
        "xT": _bf(xT),
        "fc0_wT": _bf(np.asarray(fc0_w).T),                  # [64, 256]
        "fc0_b": _f32(np.asarray(fc0_b).reshape(2, 128).T),  # [128, 2]
        "fnoW": _bf(fnoW),
        "linb": _f32(linb),
        "W0": _bf(W0.reshape(6, 128, G4)),
        "b0r": _bf(np.broadcast_to(b0, (32, G4))),
        "W1": _bf(W1.reshape(8, 128, G4)),
        "b1r": _bf(np.broadcast_to(b1, (32, G4))),
        "eye32": _bf(np.eye(32)),
        "fcoutT": _bf(fcoutT),
        "fcout_b": _f32(np.broadcast_to(np.asarray(fcout_b), (32, OUTD))),
    }


_INPUT_SPECS = [
    ("xT", (D, TB), BF16),
    ("fc0_wT", (D, W), BF16),
    ("fc0_b", (128, 2), F32),
    ("fnoW", (4, 2, 2, 128, 128), BF16),
    ("linb", (4, 2, 128), F32),
    ("W0", (6, 128, G4), BF16),
    ("b0r", (32, G4), BF16),
    ("W1", (8, 128, G4), BF16),
    ("b1r", (32, G4), BF16),
    ("eye32", (32, 32), BF16),
    ("fcoutT", (128, 4, OUTD), BF16),
    ("fcout_b", (32, OUTD), F32),
]


def build_kernel():
    nc = bacc.Bacc("TRN2", target_bir_lowering=False, debug=False)
    dram = {}
    for name, shape, dt in _INPUT_SPECS:
        dram[name] = nc.dram_tensor(name, list(shape), dt, kind="ExternalInput").ap()
    out_d = nc.dram_tensor("out", [B, OUTD], F32, kind="ExternalOutput").ap()
    h_d = nc.dram_tensor("h_out", [2, B, H], F32, kind="ExternalOutput").ap()
    c_d = nc.dram_tensor("c_out", [2, B, H], F32, kind="ExternalOutput").ap()
    with tile.TileContext(nc) as tc:
        _emit(tc, dram, out_d, h_d, c_d)
    nc.compile()
    return nc


def _emit(tc, dram, out_d, h_d, c_d):
    nc = tc.nc
    PE = mybir.EngineType.PE

    with ExitStack() as ctx:
        wp = ctx.enter_context(tc.tile_pool(name="wp", bufs=1))
        sp = ctx.enter_context(tc.tile_pool(name="sp", bufs=1))
        wk = ctx.enter_context(tc.tile_pool(name="wk", bufs=3))

        # ---- persistent weights in SBUF ----
        fc0w_sb = wp.tile([D, W], BF16)
        nc.sync.dma_start(fc0w_sb[:], dram["fc0_wT"])
        fc0b_sb = wp.tile([128, 2], F32)
        nc.sync.dma_start(fc0b_sb[:], dram["fc0_b"])
        fnoW_sb = wp.tile([128, 4, 2, 2, 128], BF16)
        nc.sync.dma_start(fnoW_sb[:], dram["fnoW"].rearrange("l k m p c -> p l k m c"))
        linb_sb = wp.tile([128, 4, 2], F32)
        nc.sync.dma_start(linb_sb[:], dram["linb"].rearrange("l m p -> p l m"))
        W0_sb = wp.tile([128, 6, G4], BF16)
        nc.sync.dma_start(W0_sb[:], dram["W0"].rearrange("k p g -> p k g"))
        W1_sb = wp.tile([128, 8, G4], BF16)
        nc.sync.dma_start(W1_sb[:], dram["W1"].rearrange("k p g -> p k g"))
        b0r_sb = wp.tile([32, G4], BF16)
        nc.sync.dma_start(b0r_sb[:], dram["b0r"])
        b1r_sb = wp.tile([32, G4], BF16)
        nc.sync.dma_start(b1r_sb[:], dram["b1r"])
        eye_sb = wp.tile([32, 32], BF16)
        nc.sync.dma_start(eye_sb[:], dram["eye32"])
        fcoutw_sb = wp.tile([128, 4, OUTD], BF16)
        nc.sync.dma_start(fcoutw_sb[:], dram["fcoutT"])
        fcb_sb = wp.tile([32, OUTD], F32)
        nc.sync.dma_start(fcb_sb[:], dram["fcout_b"])

        # FNO ping-pong activations, transposed: [128, kt, TB+pad]
        PAD = 512
        fA = wp.tile([128, 2, TB + PAD], BF16)
        fB = wp.tile([128, 2, TB + PAD], BF16)

        # Prime the ACT engine's vector clock on the DMA-loaded bias tiles so
        # later Activations (whose ISA slot fits only one sync wait) never
        # need a DMA wait in addition to their PE wait.
        prime1 = wk.tile([128, 1], F32, tag="prime1")
        nc.scalar.activation(prime1[:], fc0b_sb[:, 0:1], AF.Identity)
        prime2 = wk.tile([128, 1], F32, tag="prime2")
        nc.scalar.activation(prime2[:], linb_sb[:, 0, 0:1], AF.Identity)

        # ---- fc0 + FNO stack (own psum pool scope) ----
        with tc.tile_pool(name="fpp", bufs=2, space="PSUM") as fpp, \
             tc.tile_pool(name="xp", bufs=4) as xp:
            for m in range(2):
                for nb in range(0, NCHUNK, NBLK):
                    psf = fpp.tile([128, NBLK, 512], F32, tag="fno_ps")
                    for n in range(nb, nb + NBLK):
                        xt = xp.tile([D, 512], BF16, tag="xt")
                        nc.sync.dma_start(xt[:], dram["xT"][:, ts(n, 512)])
                        nc.tensor.matmul(
                            psf[:, n - nb, :],
                            lhsT=fc0w_sb[:, ts(m, 128)],
                            rhs=xt[:],
                            start=True, stop=True,
                        )
                    nc.scalar.activation(
                        fA[:, m, ds(nb * 512, NBLK * 512)], psf[:, :, :],
                        AF.Identity, bias=fc0b_sb[:, ds(m, 1)],
                    )
            bufs = [fA, fB]
            for l in range(4):
                src, dst = bufs[l % 2], bufs[(l + 1) % 2]
                for m in range(2):
                    for nb in range(0, NCHUNK, NBLK):
                        psf = fpp.tile([128, NBLK, 512], F32, tag="fno_ps")
                        for k in range(2):
                            for n in range(nb, nb + NBLK):
                                nc.tensor.matmul(
                                    psf[:, n - nb, :],
                                    lhsT=fnoW_sb[:, l, k, m, :],
                                    rhs=src[:, k, ts(n, 512)],
                                    start=(k == 0), stop=(k == 1),
                                )
                        nc.scalar.activation(
                            dst[:, m, ds(nb * 512, NBLK * 512)], psf[:, :, :],
                            AF.Gelu, bias=linb_sb[:, l, ds(m, 1)],
                        )
        fno_out = fA  # after 4 layers: back to fA

        # ---- LSTM scan ----
        spp = ctx.enter_context(tc.tile_pool(name="spp", bufs=2, space="PSUM"))

        c_sb = sp.tile([128, 2, 128], F32)          # [32q+b, layer, h_lo]
        nc.gpsimd.memset(c_sb[:], 0.0)
        hT0 = sp.tile([128, 2, 4, 32], BF16)        # [h_lo, parity, kt, b]
        nc.gpsimd.memset(hT0[:], 0.0)
        hT1 = sp.tile([128, 2, 4, 32], BF16)
        nc.gpsimd.memset(hT1[:], 0.0)
        xstage = sp.tile([128, 2, 2, 256], BF16)    # [p, buf, kt, 8ticks*32]

        hf_out = [None, None]

        def layer_tick(l, x_tiles, h_prev, h_par, want_f32):
            """One LSTM cell step for layer l.
            x_tiles: lhsT APs [*,32] for the input contribution k-tiles.
            h_prev: [128, 4, 32] bf16 lhsT AP of h_{t-1}; h_par: parity to write.
            """
            Wsb = W0_sb if l == 0 else W1_sb
            brep = b0r_sb if l == 0 else b1r_sb
            hT = hT0 if l == 0 else hT1
            nkx = len(x_tiles)
            nk = nkx + 5
            ps = spp.tile([128, 512], F32, tag=f"gps{l}")
            for ki in range(nk):
                for cg in range(4):
                    o_ap = ps[ds(32 * cg, 32), :]
                    gsl = ds(cg * 512, 512)
                    tp = (0, 32 * cg)
                    if ki == 0:
                        nc.tensor.matmul(o_ap, lhsT=eye_sb[:], rhs=brep[:, gsl],
                                         start=True, stop=False, tile_position=tp)
                    elif ki <= nkx:
                        nc.tensor.matmul(o_ap, lhsT=x_tiles[ki - 1],
                                         rhs=Wsb[:, ki - 1, gsl],
                                         start=False, stop=False, tile_position=tp)
                    else:
                        k = ki - 1  # row-tile index in Wsb (h rows follow x rows)
                        nc.tensor.matmul(o_ap, lhsT=h_prev[:, k - nkx, :],
                                         rhs=Wsb[:, k, gsl],
                                         start=False, stop=(ki == nk - 1),
                                         tile_position=tp)
            ifo = wk.tile([128, 384], F32, tag=f"ifo{l}")
            g = wk.tile([128, 128], F32, tag=f"g{l}")
            nc.scalar.activation(ifo[:], ps[:, 0:384], AF.Sigmoid)
            nc.scalar.activation(g[:], ps[:, 384:512], AF.Tanh)
            t1 = wk.tile([128, 128], F32, tag=f"t1{l}")
            t2 = wk.tile([128, 128], F32, tag=f"t2{l}")
            nc.vector.tensor_tensor(t1[:], ifo[:, 0:128], g[:], OP.mult)
            nc.vector.tensor_tensor(t2[:], ifo[:, 128:256], c_sb[:, l, :], OP.mult)
            nc.vector.tensor_tensor(c_sb[:, l, :], t1[:], t2[:], OP.add)
            tanhc = wk.tile([128, 128], F32, tag=f"th{l}")
            nc.scalar.activation(tanhc[:], c_sb[:, l, :], AF.Tanh)
            hbf = wk.tile([128, 128], BF16, tag=f"hb{l}")
            nc.vector.tensor_tensor(hbf[:], ifo[:, 256:384], tanhc[:], OP.mult)
            nc.sync.dma_start_transpose(hT[:, h_par], hbf[:])
            if want_f32:
                hf = sp.tile([128, 128], F32, tag=f"hf{l}")
                nc.vector.tensor_tensor(hf[:], ifo[:, 256:384], tanhc[:], OP.mult)
                hf_out[l] = hf

        def x_static(i):
            return [fno_out[:, k, ds(i * 32, 32)] for k in range(2)]

        def l0_tick(i, x_tiles):
            # L0 step i: reads hT0[(i-1)%2], writes parity i%2
            layer_tick(0, x_tiles, hT0[:, (i + 1) % 2], i % 2, i == T - 1)

        def l1_tick(j):
            # L1 step j: x-side = hT0[j%2], h-side = hT1[(j-1)%2], writes j%2
            x_tiles = [hT0[:, j % 2, k, :] for k in range(4)]
            layer_tick(1, x_tiles, hT1[:, (j + 1) % 2], j % 2, j == T - 1)

        if T > UNROLL + 1:
            n_loop = ((T - 1) // UNROLL) * UNROLL
            if n_loop + 1 > T:
                n_loop -= UNROLL
        else:
            n_loop = 0

        l0_tick(0, x_static(0))
        if n_loop > 0:
            # pre-stage x for loop ticks 1..16 (buf 0: ticks 1-8, buf 1: 9-16)
            nc.gpsimd.dma_start(xstage[:, 0], fno_out[:, :, ds(1 * 32, 256)])
            nc.gpsimd.dma_start(xstage[:, 1], fno_out[:, :, ds(9 * 32, 256)])
            with tc.For_i(1, 1 + n_loop, UNROLL, hint_engines=(PE,)) as iv:
                for u in range(UNROLL):
                    par = (1 + u) % 2       # parity of tick i = iv+u (iv odd)
                    opar = (par + 1) % 2
                    sb = 0 if u < 8 else 1
                    x_tiles = [xstage[:, sb, k, ds((u % 8) * 32, 32)] for k in range(2)]
                    # L0 step i: reads hT0[opar], writes par
                    layer_tick(0, x_tiles, hT0[:, opar], par, False)
                    # L1 step i-1: x = hT0[opar], h = hT1[par], writes opar
                    layer_tick(1, [hT0[:, opar, k, :] for k in range(4)],
                               hT1[:, par], opar, False)
                    if u == 7:
                        nc.gpsimd.dma_start(xstage[:, 0],
                                            fno_out[:, :, ds((iv + 16) * 32, 256)])
                    if u == 15:
                        nc.gpsimd.dma_start(xstage[:, 1],
                                            fno_out[:, :, ds((iv + 24) * 32, 256)])
        for i in range(1 + n_loop, T):
            l0_tick(i, x_static(i))
            l1_tick(i - 1)
        l1_tick(T - 1)

        # ---- outputs ----
        for l in range(2):
            cf = wk.tile([128, 128], F32, tag="cf")
            nc.vector.tensor_copy(cf[:], c_sb[:, l, :])
            for q in range(4):
                nc.sync.dma_start(h_d[l][:, ds(128 * q, 128)],
                                  hf_out[l][ds(32 * q, 32), :])
                nc.sync.dma_start(c_d[l][:, ds(128 * q, 128)],
                                  cf[ds(32 * q, 32), :])

        # out = h2 @ fcout_w.T + fcout_b   (h2 as bf16 hT1, fp32 psum accum)
        po = spp.tile([32, OUTD], F32, tag="po")
        h2T = hT1[:, (T - 1) % 2]
        for k in range(4):
            nc.tensor.matmul(po[:], lhsT=h2T[:, k, :], rhs=fcoutw_sb[:, k, :],
                             start=(k == 0), stop=(k == 3))
        out_sb = wk.tile([32, OUTD], F32, tag="osb")
        nc.vector.tensor_tensor(out_sb[:], po[:], fcb_sb[:], OP.add)
        nc.sync.dma_start(out_d, out_sb[:])


_CACHE = {}


def _get_nc():
    if "nc" not in _CACHE:
        _CACHE["nc"] = build_kernel()
    return _CACHE["nc"]


def kernel(**inputs):
    prep = prepare_inputs(**inputs)
    nc = _get_nc()
    in_map = {k: prep[k] for k, _, _ in _INPUT_SPECS}
    trace = os.environ.get("KERNEL_TRACE", "0") == "1"
    res = run_bass_kernel_spmd(nc, [dict(in_map) for _ in range(N_CORES)],
                               core_ids=list(range(N_CORES)), trace=trace)
    _CACHE["last_res"] = res
    r0 = res.results[0]
    return (
        np.asarray(r0["out"], dtype=np.float32),
        np.asarray(r0["h_out"], dtype=np.float32),
        np.asarray(r0["c_out"], dtype=np.float32),
    )
